# revision 1
# baseline (speedup 1.0000x reference)
"""Trainium2 Bass kernel for nn_NoGraphLayer (single-query neighbor attention + FFN).

Full (unsharded) inputs in, full output out. Internally: pure data-parallel
shard of the anchor dim B=32768 across 8 NeuronCores (4096 anchors each).

Per-core layout strategy:
  - Process 32 blocks of 128 anchors. Block's x_nei tile is [128 b, 32*256]
    (b on partitions, (k, d) on free) -- one contiguous 4 MiB DMA.
  - Per k: PE-transpose x_k -> xT [d, b] (fp32r), projections use xT as the
    self-loading stationary operand with W_k/W_v moving (fp32r, 1 cyc/row),
    giving Kv/Vv in natural [b, d'] layout in PSUM.
  - scores/attention stay per-partition (b): DVE mul with Q tile reused
    across k (k-major rows), segmented free-dim reduce over dk.
  - attn@V: DVE mul with attn broadcast over dk, then PE identity-matmul
    accumulation over k into PSUM.
  - LayerNorm g/b folded host-side into the adjacent matmul weights/bias rows.
  - rsqrt computed as exp(-0.5*ln(.)) to keep all ACT transcendentals except
    gelu in one table set (natural_log_exp_and_others).
"""

import math
from contextlib import ExitStack

import numpy as np

import concourse.bass as bass
import concourse.tile as tile
from concourse import bacc
from concourse import mybir

F32 = mybir.dt.float32
F32R = mybir.dt.float32r
AX = mybir.AxisListType
ALU = mybir.AluOpType
ACT = mybir.ActivationFunctionType

B, K, D, H, FF = 32768, 32, 256, 8, 1024
DK = D // H
P = 128
NCORES = 8
BC = B // NCORES            # anchors per core
NBLK = BC // P              # 32 blocks per core
KB = 2                      # k's per psum projection batch (1 bank)
TB = 4                      # k's per transpose/copy batch (2 banks)
EPS = 1e-5
CPACK_COLS = 6144 + 128
VPACK_COLS = 128 + 256 + 1024 + 256


def _r(ap):  # retained for ad-hoc experiments
    return ap.bitcast(F32R)


def emit_layer(tc, io, n_blocks):
    nc = tc.nc
    with ExitStack() as ctx:
        const = ctx.enter_context(tc.tile_pool(name="const", bufs=1))
        xpool = ctx.enter_context(tc.tile_pool(name="xin", bufs=2))
        xtp = ctx.enter_context(tc.tile_pool(name="xt", bufs=2))
        work = ctx.enter_context(tc.tile_pool(name="work", bufs=2))
        work1 = ctx.enter_context(tc.tile_pool(name="work1", bufs=1))
        prod = ctx.enter_context(tc.tile_pool(name="prod", bufs=2))
        ps_t = ctx.enter_context(tc.tile_pool(name="pst", bufs=2, space="PSUM"))
        ps_kv = ctx.enter_context(tc.tile_pool(name="pskv", bufs=3, space="PSUM"))
        ps_u = ctx.enter_context(tc.tile_pool(name="psu", bufs=1, space="PSUM"))

        # ---- constants (3 packed DMAs to stay under per-inst wait limits) ----
        cpack = const.tile([P, CPACK_COLS], F32R, tag="c_pack")
        nc.sync.dma_start(cpack[:], io["cpack"])
        idn_f = const.tile([P, P], F32, tag="c_identf")
        nc.sync.dma_start(idn_f[:], io["identf"])
        vpack = const.tile([1, VPACK_COLS], F32R, tag="c_vpack")
        nc.sync.dma_start(vpack[:], io["vpack"])

        def cslice(off, n, view=None):
            ap = cpack[:, off:off + n]
            return ap if view is None else ap.rearrange("p (kt n) -> p kt n", n=view)

        wq = cslice(0, 512, D)
        wk = cslice(512, 1024 - 512, D)
        wv = cslice(1024, 512, D)
        wo = cslice(1536, 512, D)
        f1w = cslice(2048, 2048, FF)
        f2w = cslice(4096, 2048, D)
        idn = cpack[:, 6144:6144 + P]
        ones1 = vpack[:, 0:P]
        bq = vpack[:, P:P + D]
        b1 = vpack[:, P + D:P + D + FF]
        b2 = vpack[:, P + D + FF:P + D + FF + D]

        def layernorm_normed(x_t, out_t):
            # out = (x - mu) * rsqrt(var + eps)   (g/b folded into next matmul)
            s = work.tile([P, 1], F32, tag="ln_s")
            nc.vector.tensor_reduce(s[:], x_t[:], axis=AX.X, op=ALU.add)
            mu = work.tile([P, 1], F32, tag="ln_mu")
            nc.scalar.mul(mu[:], s[:], 1.0 / D)
            sq = work.tile([P, D], F32, tag="ln_sq")
            nc.scalar.activation(sq[:], x_t[:], ACT.Square)
            ssq = work.tile([P, 1], F32, tag="ln_ssq")
            nc.vector.tensor_reduce(ssq[:], sq[:], axis=AX.X, op=ALU.add)
            # var = ssq/D - mu^2 ; rsqrt(var+eps) = exp(-0.5*ln(var+eps))
            mu2 = work.tile([P, 1], F32, tag="ln_mu2")
            nc.vector.tensor_mul(mu2[:], mu[:], mu[:])
            vb = work.tile([P, 1], F32, tag="ln_vb")
            nc.vector.scalar_tensor_tensor(
                out=vb[:], in0=ssq[:], scalar=1.0 / D, in1=mu2[:],
                op0=ALU.mult, op1=ALU.subtract,
            )
            ve = work.tile([P, 1], F32, tag="ln_ve")
            nc.vector.tensor_scalar(
                out=ve[:], in0=vb[:], scalar1=EPS, scalar2=None, op0=ALU.add)
            lnv = work.tile([P, 1], F32, tag="ln_lnv")
            nc.scalar.activation(lnv[:], ve[:], ACT.Ln)
            rs = work.tile([P, 1], F32, tag="ln_rs")
            nc.scalar.activation(rs[:], lnv[:], ACT.Exp, scale=-0.5)
            nc.vector.tensor_scalar(
                out=out_t[:], in0=x_t[:], scalar1=mu[:], scalar2=rs[:],
                op0=ALU.subtract, op1=ALU.mult,
            )

        def transpose_to(src_ap, n128, dst_tile, dst_off, rmode=True):
            # src_ap: [P, n128*128] sbuf; transpose each 128-half via PE,
            # batch through one psum tile, single ACT copy to dst.
            # rmode: fp32r path (src/psum f32r); else fp32 src, f32r dst cast.
            if rmode:
                tp = ps_t.tile([P, TB * D], F32R, tag="pt")
                ident = idn
            else:
                tp = ps_t.tile([P, TB * D], F32, tag="pt")
                ident = idn_f[:]
            for j in range(n128):
                nc.tensor.transpose(
                    tp[:, j * P:(j + 1) * P],
                    src_ap[:, j * P:(j + 1) * P],
                    ident,
                )
            nc.scalar.copy(dst_tile[:, dst_off:dst_off + n128 * P], tp[:, : n128 * P])

        for i in range(n_blocks):
            xa = xpool.tile([P, D], F32, tag="xa")
            nc.sync.dma_start(xa[:], io["x_anc"][i * P:(i + 1) * P, :])
            xn = xpool.tile([P, K * D], F32R, tag="xn")
            nc.sync.dma_start(xn[:], io["x_nei"][i * P:(i + 1) * P, :])

            # ---- LN1 + Q ----
            lnx = work1.tile([P, D], F32R, tag="lnx")
            layernorm_normed(xa, lnx)
            lnxT = work1.tile([P, D], F32R, tag="lnxT")
            transpose_to(lnx, 2, lnxT, 0)
            qp = ps_kv.tile([P, KB * D], F32, tag="pskv")
            for kt in range(2):
                nc.tensor.matmul(qp[:, 0:D], lnxT[:, kt * P:(kt + 1) * P],
                                 wq[:, kt, :], start=(kt == 0), stop=False)
            nc.tensor.matmul(qp[:, 0:D], ones1, bq,
                             start=False, stop=True)
            qs = work.tile([P, D], F32, tag="qs")
            nc.scalar.copy(qs[:], qp[:, 0:D])

            # ---- pass 1: transpose x_k, Kv, scores ----
            xT = xtp.tile([P, K, D], F32R, tag="xT")
            scoresN = work1.tile([P, K * H], F32, tag="scores")
            for tb in range(K // TB):  # transpose batches of TB k's
                transpose_to(xn[:, tb * TB * D:(tb + 1) * TB * D], TB * 2,
                             xT.rearrange("p k n -> p (k n)"), tb * TB * D)
            for kb in range(K // KB):
                kv = ps_kv.tile([P, KB * D], F32, tag="pskv")
                for kk in range(KB):
                    k = kb * KB + kk
                    for kt in range(2):
                        nc.tensor.matmul(
                            kv[:, kk * D:(kk + 1) * D],
                            xT[:, k, kt * P:(kt + 1) * P],
                            wk[:, kt, :],
                            start=(kt == 0), stop=(kt == 1),
                        )
                pr = prod.tile([P, KB * D], F32, tag="prodS")
                nc.vector.tensor_mul(
                    pr.rearrange("p (k n) -> p k n", n=D),
                    kv.rearrange("p (k n) -> p k n", n=D),
                    qs.rearrange("p (o n) -> p o n", o=1).to_broadcast((P, KB, D)),
                )
                nc.vector.tensor_reduce(
                    scoresN.rearrange("p (k h) -> p k h", h=H)[:, kb * KB:(kb + 1) * KB, :],
                    pr.rearrange("p (k h dk) -> p k h dk", h=H, dk=DK),
                    axis=AX.X, op=ALU.add,
                )

            # ---- softmax over k (no max-subtraction; scores are O(1)) ----
            e = work1.tile([P, K * H], F32, tag="e")
            nc.scalar.activation(e[:], scoresN[:], ACT.Exp)
            z = work.tile([P, H], F32, tag="z")
            nc.vector.tensor_reduce(
                z[:], e.rearrange("p (k h) -> p h k", h=H), axis=AX.X, op=ALU.add)
            zr = work.tile([P, H], F32, tag="zr")
            nc.vector.reciprocal(zr[:], z[:])
            attn = work1.tile([P, K * H], F32, tag="attn")
            nc.vector.tensor_mul(
                attn.rearrange("p (k h) -> p k h", h=H),
                e.rearrange("p (k h) -> p k h", h=H),
                zr.rearrange("p (o h) -> p o h", o=1).to_broadcast((P, K, H)),
            )

            # ---- pass 2: Vv, weighted sum over k ----
            up = ps_u.tile([P, D], F32, tag="psu")
            for kb in range(K // KB):
                vv = ps_kv.tile([P, KB * D], F32, tag="pskv")
                for kk in range(KB):
                    k = kb * KB + kk
                    for kt in range(2):
                        nc.tensor.matmul(
                            vv[:, kk * D:(kk + 1) * D],
                            xT[:, k, kt * P:(kt + 1) * P],
                            wv[:, kt, :],
                            start=(kt == 0), stop=(kt == 1),
                        )
                pv = prod.tile([P, KB * D], F32R, tag="prodV")
                nc.vector.tensor_mul(
                    pv.rearrange("p (k h dk) -> p k h dk", h=H, dk=DK),
                    vv.rearrange("p (k h dk) -> p k h dk", h=H, dk=DK),
                    attn.rearrange("p (k h) -> p k h", h=H)[:, kb * KB:(kb + 1) * KB, :]
                        .to_broadcast((P, KB, H, DK)),
                )
                for kk in range(KB):
                    k = kb * KB + kk
                    nc.tensor.matmul(
                        up[:], idn, pv[:, kk * D:(kk + 1) * D],
                        start=(k == 0), stop=(k == K - 1),
                    )

            # ---- W_o + residual ----
            us = work1.tile([P, D], F32R, tag="us")
            nc.scalar.copy(us[:], up[:])
            uT = work1.tile([P, D], F32R, tag="uT")
            transpose_to(us, 2, uT, 0)
            ao = ps_kv.tile([P, KB * D], F32, tag="pskv")
            for kt in range(2):
                nc.tensor.matmul(ao[:, 0:D], uT[:, kt * P:(kt + 1) * P],
                                 wo[:, kt, :], start=(kt == 0), stop=(kt == 1))
            xs = work.tile([P, D], F32, tag="xs")
            nc.vector.tensor_add(xs[:], ao[:, 0:D], xa[:])

            # ---- LN2 + FF ----
            hs = work1.tile([P, D], F32R, tag="hs")
            layernorm_normed(xs, hs)
            hT = work1.tile([P, D], F32R, tag="hT")
            transpose_to(hs, 2, hT, 0)
            ffg = work1.tile([P, FF], F32R, tag="ffg")
            for nh in range(2):
                fp = ps_kv.tile([P, KB * D], F32, tag="pskv")
                for kt in range(2):
                    nc.tensor.matmul(fp[:], hT[:, kt * P:(kt + 1) * P],
                                     f1w[:, kt, nh * 512:(nh + 1) * 512],
                                     start=(kt == 0), stop=False)
                nc.tensor.matmul(fp[:], ones1, b1[:, nh * 512:(nh + 1) * 512],
                                 start=False, stop=True)
                nc.scalar.activation(ffg[:, nh * 512:(nh + 1) * 512], fp[:], ACT.Gelu)
            fgT = work1.tile([P, FF], F32R, tag="fgT")
            for q in range(2):
                transpose_to(ffg[:, q * 512:(q + 1) * 512], 4, fgT, q * 512)
            f2p = ps_kv.tile([P, KB * D], F32, tag="pskv")
            for kt in range(8):
                nc.tensor.matmul(f2p[:, 0:D], fgT[:, kt * P:(kt + 1) * P],
                                 f2w[:, kt, :], start=(kt == 0), stop=False)
            nc.tensor.matmul(f2p[:, 0:D], ones1, b2,
                             start=False, stop=True)
            outs = work.tile([P, D], F32, tag="outs")
            nc.vector.tensor_add(outs[:], f2p[:, 0:D], xs[:])
            nc.sync.dma_start(io["out"][i * P:(i + 1) * P, :], outs[:])


_ACT_TABLES_PATCHED = False


def _patch_act_tables():
    # Bias bacc's act-table chooser: Ln and Exp both resolve to the
    # natural_log_exp_and_others set (one resident table set for LN-rsqrt
    # and softmax), instead of bouncing between natural_log and
    # exp_and_others every block. Set ids stay untouched.
    global _ACT_TABLES_PATCHED
    if _ACT_TABLES_PATCHED:
        return
    import concourse.bacc as _bacc_mod
    _orig = _bacc_mod.get_activation_tables

    def patched(arch):
        t = dict(_orig(arch))
        exp_t = mybir.ActivationFunctionType.Exp
        ln_t = mybir.ActivationFunctionType.Ln
        for name, fns in t.items():
            if name != "natural_log_exp_and_others" and (
                    exp_t in fns or ln_t in fns):
                t[name] = fns - {exp_t, ln_t}
        return t

    _bacc_mod.get_activation_tables = patched
    _ACT_TABLES_PATCHED = True


def build_bass(n_blocks=NBLK, bc=BC):
    _patch_act_tables()
    nc = bacc.Bacc("TRN2", target_bir_lowering=False, debug=False,
                   num_devices=NCORES)
    io = {}
    io["x_anc"] = nc.dram_tensor("x_anc", [bc, D], F32, kind="ExternalInput").ap()
    io["x_nei"] = nc.dram_tensor("x_nei", [bc, K * D], F32R, kind="ExternalInput").ap()
    io["cpack"] = nc.dram_tensor("cpack", [P, CPACK_COLS], F32R, kind="ExternalInput").ap()
    io["identf"] = nc.dram_tensor("identf", [P, P], F32, kind="ExternalInput").ap()
    io["vpack"] = nc.dram_tensor("vpack", [1, VPACK_COLS], F32R, kind="ExternalInput").ap()
    io["out"] = nc.dram_tensor("out", [bc, D], F32, kind="ExternalOutput").ap()
    with tile.TileContext(nc) as tc:
        emit_layer(tc, io, n_blocks)
    nc.compile()
    return nc


_CACHED_NC = None


def make_in_maps(inputs):
    f = np.float32
    x_anc = np.ascontiguousarray(inputs["x_anc"], dtype=f)
    x_nei = np.ascontiguousarray(inputs["x_nei"], dtype=f)
    ln1_g = np.asarray(inputs["ln1_g"], f)
    ln1_b = np.asarray(inputs["ln1_b"], f)
    ln2_g = np.asarray(inputs["ln2_g"], f)
    ln2_b = np.asarray(inputs["ln2_b"], f)
    sc = f(1.0 / math.sqrt(DK))
    # fold LN gains/biases + score scale into the adjacent matmuls
    wq_f = np.ascontiguousarray((ln1_g[:, None] * np.asarray(inputs["W_q"], f)) * sc)
    bias_q = (ln1_b @ np.asarray(inputs["W_q"], f) * sc)[None, :]
    ff1w_f = np.ascontiguousarray(ln2_g[:, None] * np.asarray(inputs["ff1_w"], f))
    bias_ff1 = (np.asarray(inputs["ff1_b"], f)
                + ln2_b @ np.asarray(inputs["ff1_w"], f))[None, :]
    bias_ff2 = np.asarray(inputs["ff2_b"], f)[None, :]

    def kt_pack(w, ncols):  # [kt*128, n] -> [128, kt*n]
        ktn = w.shape[0] // P
        return w.reshape(ktn, P, ncols).transpose(1, 0, 2).reshape(P, ktn * ncols)

    cpack = np.concatenate([
        kt_pack(wq_f, D), kt_pack(np.asarray(inputs["W_k"], f), D),
        kt_pack(np.asarray(inputs["W_v"], f), D),
        kt_pack(np.asarray(inputs["W_o"], f), D),
        kt_pack(ff1w_f, FF), kt_pack(np.asarray(inputs["ff2_w"], f), D),
        np.eye(P, dtype=f),
    ], axis=1)
    vpack = np.concatenate([
        np.ones((1, P), f), bias_q, bias_ff1, bias_ff2], axis=1)
    shared = {
        "cpack": np.ascontiguousarray(cpack),
        "identf": np.eye(P, dtype=f),
        "vpack": np.ascontiguousarray(vpack.astype(f)),
    }
    in_maps = []
    for c in range(NCORES):
        sl = slice(c * BC, (c + 1) * BC)
        m = dict(shared)
        m["x_anc"] = np.ascontiguousarray(x_anc[sl])
        m["x_nei"] = np.ascontiguousarray(x_nei[sl].reshape(BC, K * D))
        in_maps.append(m)
    return in_maps


def get_nc():
    global _CACHED_NC
    if _CACHED_NC is None:
        _CACHED_NC = build_bass()
    return _CACHED_NC


def kernel(**inputs):
    from concourse.bass_utils import run_bass_kernel_spmd

    in_maps = make_in_maps(inputs)
    res = run_bass_kernel_spmd(get_nc(), in_maps, core_ids=list(range(NCORES)))
    return np.concatenate([res.results[c]["out"] for c in range(NCORES)], axis=0)



# revision 2
# speedup vs baseline: 1.0156x; 1.0156x over previous
"""Trainium2 Bass kernel v2 for nn_NoGraphLayer (single-query neighbor attention + FFN).

Full inputs in, full output out; data-parallel B across 8 cores (4096 anchors each,
32 blocks of 128).

v2 strategy (vs fp32r baseline):
  - x_nei is cast fp32->bf16 during the HBM load (gpsimd SWDGE casting DMA), so
    PE transposes run in bf16 (FWL weight loads, 1 cyc/row).
  - K and V projections fused into ONE DoubleRow fp8 matmul per neighbor
    (contraction 256 in a single pass). W_k/W_v are 16x-scaled host-side so fp8
    avoids the subnormal range; 1/16 folded into W_q (K side) and W_o (V side).
  - Transposed x tiles are copied PSUM->SBUF as fp8e4 in the DR-interleaved
    layout [ki, k, ko, b] by ACT (the copy that had to happen anyway).
  - DVE softmax path in bf16 with layouts keeping a packed innermost dim:
    scores/e in [p,(h,k)], Vv/pv in [p,(h dk,k)] so e-broadcast multiplies and
    k-reductions stay 2x/4x eligible.
  - attn@V summed on DVE (contiguous inner-k reduce), not PE identity-matmuls.
  - Vv drained PSUM->SBUF bf16 by ACT per 4-k group; Kv consumed directly from
    PSUM by the scores multiply (DVE), so kv PSUM tiles free quickly.
  - LayerNorm gains/biases folded into adjacent weights; rsqrt via exp(-.5 ln .)
    (resident act table); gelu keeps the exact ACT Gelu.
"""

import math
from contextlib import ExitStack

import numpy as np

import concourse.bass as bass
import concourse.tile as tile
from concourse import bacc
from concourse import mybir

F32 = mybir.dt.float32
BF16 = mybir.dt.bfloat16
FP8 = mybir.dt.float8e4
AX = mybir.AxisListType
ALU = mybir.AluOpType
ACT = mybir.ActivationFunctionType
DR = mybir.MatmulPerfMode.DoubleRow

B, K, D, H, FF = 32768, 32, 256, 8, 1024
DK = D // H
P = 128
NCORES = 8
BC = B // NCORES
NBLK = BC // P
EPS = 1e-5
WS = 16.0                      # host-side scale on W_k/W_v for fp8 range
KG = 4                         # k's per kv psum tile / drain group
TB = 8                         # transposes per psum batch (= 4 k's)

# const pack layouts (free-dim columns)
BPACK_COLS = 2 * D + 2 * D + 2 * FF + 8 * D + P      # wq, wo, f1w, f2w, identb
FPACK_COLS = D                                        # bq replicated
VPACK_COLS = P + FF + D                               # ones, b1, b2 (bf16 rows)
QPACK_COLS = 2 * 2 * D                                # wkv fp8 [2, 512]


def emit_layer(tc, io, n_blocks):
    nc = tc.nc
    with ExitStack() as ctx:
        ctx.enter_context(nc.allow_low_precision(
            reason="bf16 softmax path is within the 2e-2 gate"))
        const = ctx.enter_context(tc.tile_pool(name="const", bufs=1))
        xpool = ctx.enter_context(tc.tile_pool(name="xin", bufs=2))
        xtp = ctx.enter_context(tc.tile_pool(name="xt", bufs=2))
        work = ctx.enter_context(tc.tile_pool(name="work", bufs=2))
        big = ctx.enter_context(tc.tile_pool(name="big", bufs=2))
        t1p = ctx.enter_context(tc.tile_pool(name="t1p", bufs=1))
        vvp = ctx.enter_context(tc.tile_pool(name="vvp", bufs=2))
        ps_t = ctx.enter_context(tc.tile_pool(name="pst", bufs=2, space="PSUM"))
        ps_kv = ctx.enter_context(tc.tile_pool(name="pskv", bufs=1, space="PSUM"))
        ps_f = ctx.enter_context(tc.tile_pool(name="psf", bufs=2, space="PSUM"))

        bpack = const.tile([P, BPACK_COLS], BF16, tag="c_bpack")
        nc.sync.dma_start(bpack[:], io["bpack"])
        qpack = const.tile([P, QPACK_COLS], FP8, tag="c_qpack")
        nc.sync.dma_start(qpack[:], io["qpack"])
        fpack = const.tile([P, FPACK_COLS], F32, tag="c_fpack")
        nc.sync.dma_start(fpack[:], io["fpack"])
        vpack = const.tile([1, VPACK_COLS], BF16, tag="c_vpack")
        nc.sync.dma_start(vpack[:], io["vpack"])

        off = [0]

        def bslice(n, view=None):
            ap = bpack[:, off[0]:off[0] + n]
            off[0] += n
            return ap if view is None else ap.rearrange("p (kt n) -> p kt n", n=view)

        wq = bslice(2 * D, D)          # [p, 2, 256]
        wo = bslice(2 * D, D)
        f1w = bslice(2 * FF, FF)       # [p, 2, 1024]
        f2w = bslice(8 * D, D)         # [p, 8, 256]
        identb = bslice(P)
        wkv = qpack[:].rearrange("p (ko n) -> p ko n", n=2 * D)  # [p, 2, 512] fp8
        bq = fpack[:, 0:D]             # [128, 256] f32 replicated
        ones1 = vpack[:, 0:P]
        b1 = vpack[:, P:P + FF]
        b2 = vpack[:, P + FF:P + FF + D]

        def layernorm_normed(x_t, out_t, tag):
            # out = (x - mu) * rsqrt(var + eps); g/b folded into next matmul
            st6 = work.tile([P, 6], F32, tag=tag + "_st6")
            nc.vector.bn_stats(st6[:], x_t[:])
            mv = work.tile([P, 2], F32, tag=tag + "_mv")
            nc.vector.bn_aggr(mv[:], st6[:])
            ve = work.tile([P, 1], F32, tag=tag + "_ve")
            nc.vector.tensor_scalar(
                out=ve[:], in0=mv[:, 1:2], scalar1=EPS, scalar2=None, op0=ALU.add)
            lnv = work.tile([P, 1], F32, tag=tag + "_lnv")
            nc.scalar.activation(lnv[:], ve[:], ACT.Ln)
            rs = work.tile([P, 1], F32, tag=tag + "_rs")
            nc.scalar.activation(rs[:], lnv[:], ACT.Exp, scale=-0.5)
            nc.vector.tensor_scalar(
                out=out_t[:], in0=x_t[:], scalar1=mv[:, 0:1], scalar2=rs[:],
                op0=ALU.subtract, op1=ALU.mult)

        def transpose_small(src_ap, n128, dst_tile, tag):
            # src [P, n128*128] bf16 SBUF -> dst [P, n128, 128] bf16 SBUF
            tp = ps_t.tile([P, TB * P], BF16, tag="pt")
            for j in range(n128):
                nc.tensor.transpose(
                    tp[:, j * P:(j + 1) * P], src_ap[:, j * P:(j + 1) * P], identb)
            nc.scalar.copy(
                dst_tile[:].rearrange("p a b -> p (a b)"), tp[:, :n128 * P])

        def load_block(i):
            xa = xpool.tile([P, D], F32, tag="xa")
            nc.sync.dma_start(xa[:], io["x_anc"][i * P:(i + 1) * P, :])
            xnb = xpool.tile([P, K * D], BF16, tag="xnb")
            nc.gpsimd.dma_start(xnb[:], io["x_nei"][i * P:(i + 1) * P, :])
            return {"xa": xa, "xnb": xnb}

        def emit_q(st):
            # LN1 + Q -> qs (bf16, bias folded via replicated-row add)
            lnx = work.tile([P, D], BF16, tag="lnx")
            layernorm_normed(st["xa"], lnx, "ln1")
            lnxT = work.tile([P, 2, P], BF16, tag="lnxT")
            transpose_small(lnx[:], 2, lnxT, "lnxT")
            qp = ps_f.tile([P, D], F32, tag="psf")
            for kt in range(2):
                nc.tensor.matmul(qp[:], lnxT[:, kt, :], wq[:, kt, :],
                                 start=(kt == 0), stop=(kt == 1))
            qs = work.tile([P, D], BF16, tag="qs")
            nc.vector.tensor_add(qs[:], qp[:], bq)
            st["qs"] = qs

        def emit_tb(st, tb):
            # transpose batch tb of x_nei (bf16) -> xTil fp8 [ki, k, ko, b]
            if tb == 0:
                xTil = xtp.tile([P, K, 2, P], FP8, tag="xTil")
                st["xTil"] = xTil
            tp = ps_t.tile([P, TB * P], BF16, tag="pt")
            for j in range(TB):
                c = tb * TB + j
                nc.tensor.transpose(
                    tp[:, j * P:(j + 1) * P],
                    st["xnb"][:, c * P:(c + 1) * P], identb)
            ks = tb * (TB // 2)
            nc.scalar.copy(
                st["xTil"][:, ks:ks + TB // 2].rearrange("p a b c -> p (a b c)"),
                tp[:, :TB * P])

        def emit_kv_g(st, g):
            # fused K|V fp8-DR projections for KG k's + scores mul + Vv drain
            if g == 0:
                pr = big.tile([P, K, D], BF16, tag="pr")
                vvb = vvp.tile([P, K, D], BF16, tag="vvb")
                st["pr"], st["vvb"] = pr, vvb
            kvp = ps_kv.tile([P, KG, 2 * D], F32, tag="pskv")
            for j in range(KG):
                k = g * KG + j
                nc.tensor.matmul(kvp[:, j, :], st["xTil"][:, k], wkv,
                                 start=True, stop=True, perf_mode=DR)
            nc.vector.tensor_mul(
                st["pr"][:, g * KG:(g + 1) * KG, :],
                kvp[:, :, 0:D],
                st["qs"][:].rearrange("p (o n) -> p o n", o=1)
                    .to_broadcast((P, KG, D)))
            nc.scalar.copy(st["vvb"][:, g * KG:(g + 1) * KG, :],
                           kvp[:, :, D:2 * D])

        def emit_bc(st, i):
            pr, vvb, xa = st["pr"], st["vvb"], st["xa"]
            # ---- dk-reduce via TT tree (tensor_reduce is 1.3ns/elem on HW) ----
            pr4 = pr[:].rearrange("p k (h dk) -> p k h dk", h=H)
            t1 = t1p.tile([P, K, H, 16], BF16, tag="t1")
            nc.vector.tensor_add(t1[:], pr4[:, :, :, 0:16], pr4[:, :, :, 16:32])
            t2 = work.tile([P, K, H, 8], BF16, tag="t2")
            nc.vector.tensor_add(t2[:], t1[:, :, :, 0:8], t1[:, :, :, 8:16])
            t3 = work.tile([P, K, H, 4], BF16, tag="t3")
            nc.vector.tensor_add(t3[:], t2[:, :, :, 0:4], t2[:, :, :, 4:8])
            t4 = work.tile([P, K, H, 2], BF16, tag="t4")
            nc.vector.tensor_add(t4[:], t3[:, :, :, 0:2], t3[:, :, :, 2:4])
            scoresN = work.tile([P, K, H], BF16, tag="scores")
            nc.vector.tensor_add(
                scoresN[:].rearrange("p k (h o) -> p k h o", o=1),
                t4[:, :, :, 0:1], t4[:, :, :, 1:2])
            e = work.tile([P, K, H], BF16, tag="e")
            nc.scalar.activation(
                e[:].rearrange("p k h -> p (k h)"),
                scoresN[:].rearrange("p k h -> p (k h)"), ACT.Exp)
            z = work.tile([P, H], F32, tag="z")
            nc.vector.tensor_reduce(
                z[:], e[:].rearrange("p k h -> p h k"), axis=AX.X, op=ALU.add)
            zr = work.tile([P, H], F32, tag="zr")
            nc.vector.reciprocal(zr[:], z[:])

            # ---- attn @ V: pv = e*Vv (bcast over dk), k-reduce on PE ----
            pv = vvp.tile([P, K, D], BF16, tag="pv")
            nc.vector.tensor_mul(
                pv[:].rearrange("p k (h dk) -> p k h dk", h=H),
                vvb[:].rearrange("p k (h dk) -> p k h dk", h=H),
                e[:].rearrange("p k (h o) -> p k h o", o=1)
                    .to_broadcast((P, K, H, DK)))
            upp = ps_kv.tile([P, KG, 2 * D], F32, tag="pskv")
            for k in range(K):
                nc.tensor.matmul(upp[:, 0, 0:D], identb, pv[:, k, :],
                                 start=(k == 0), stop=(k == K - 1))
            upn = work.tile([P, D], BF16, tag="upn")
            nc.vector.tensor_mul(
                upn[:].rearrange("p (h dk) -> p h dk", h=H),
                upp[:, 0, 0:D].rearrange("p (h dk) -> p h dk", h=H),
                zr[:].rearrange("p (h o) -> p h o", o=1).to_broadcast((P, H, DK)))

            # ---- W_o + residual ----
            uT = work.tile([P, 2, P], BF16, tag="uT")
            transpose_small(upn[:], 2, uT, "uT")
            aop = ps_f.tile([P, D], F32, tag="psf")
            for kt in range(2):
                nc.tensor.matmul(aop[:], uT[:, kt, :], wo[:, kt, :],
                                 start=(kt == 0), stop=(kt == 1))
            xs = work.tile([P, D], F32, tag="xs")
            nc.vector.tensor_add(xs[:], aop[:], xa[:])

            # ---- LN2 + FF ----
            hs = work.tile([P, D], BF16, tag="hs")
            layernorm_normed(xs, hs, "ln2")
            hT = work.tile([P, 2, P], BF16, tag="hT")
            transpose_small(hs[:], 2, hT, "hT")
            ffg = work.tile([P, FF], BF16, tag="ffg")
            for nh in range(2):
                fp = ps_f.tile([P, 512], F32, tag="psf")
                for kt in range(2):
                    nc.tensor.matmul(fp[:], hT[:, kt, :],
                                     f1w[:, kt, nh * 512:(nh + 1) * 512],
                                     start=(kt == 0), stop=False)
                nc.tensor.matmul(fp[:], ones1, b1[:, nh * 512:(nh + 1) * 512],
                                 start=False, stop=True)
                nc.scalar.activation(ffg[:, nh * 512:(nh + 1) * 512], fp[:],
                                     ACT.Gelu)
            fgT = work.tile([P, 8, P], BF16, tag="fgT")
            for q in range(2):
                transpose_small(ffg[:, q * 512:(q + 1) * 512], 4,
                                fgT[:, q * 4:(q + 1) * 4], "fgT")
            f2p = ps_f.tile([P, D], F32, tag="psf")
            for kt in range(8):
                nc.tensor.matmul(f2p[:], fgT[:, kt, :], f2w[:, kt, :],
                                 start=(kt == 0), stop=False)
            nc.tensor.matmul(f2p[:], ones1, b2, start=False, stop=True)
            outs = work.tile([P, D], F32, tag="outs")
            nc.vector.tensor_add(outs[:], f2p[:], xs[:])
            nc.sync.dma_start(io["out"][i * P:(i + 1) * P, :], outs[:])

        # ---- software-pipelined emission (engines execute in-order, so the
        # next block's transposes are interleaved into this block's KV groups
        # to fill PE stalls while PSUM kv tiles drain / softmax runs) ----
        sts = {0: load_block(0)}
        emit_q(sts[0])
        for tb in range(K * 2 // TB):
            emit_tb(sts[0], tb)
        for i in range(n_blocks):
            if i + 1 < n_blocks:
                sts[i + 1] = load_block(i + 1)
            for g in range(K // KG):
                emit_kv_g(sts[i], g)
                if i + 1 < n_blocks:
                    emit_tb(sts[i + 1], g)
            if i + 1 < n_blocks:
                emit_q(sts[i + 1])
            emit_bc(sts[i], i)
            del sts[i]


_ACT_TABLES_PATCHED = False


def _patch_act_tables():
    # Bias bacc's act-table chooser: Ln and Exp both resolve to
    # natural_log_exp_and_others (one resident table for LN-rsqrt and softmax);
    # only Gelu swaps tables.
    global _ACT_TABLES_PATCHED
    if _ACT_TABLES_PATCHED:
        return
    import concourse.bacc as _bacc_mod
    _orig = _bacc_mod.get_activation_tables

    def patched(arch):
        t = dict(_orig(arch))
        exp_t = mybir.ActivationFunctionType.Exp
        ln_t = mybir.ActivationFunctionType.Ln
        for name, fns in t.items():
            if name != "natural_log_exp_and_others" and (
                    exp_t in fns or ln_t in fns):
                t[name] = fns - {exp_t, ln_t}
        return t

    _bacc_mod.get_activation_tables = patched
    _ACT_TABLES_PATCHED = True


def build_bass(n_blocks=NBLK, bc=BC):
    _patch_act_tables()
    nc = bacc.Bacc("TRN2", target_bir_lowering=False, debug=False,
                   num_devices=NCORES)
    io = {}
    io["x_anc"] = nc.dram_tensor("x_anc", [bc, D], F32, kind="ExternalInput").ap()
    io["x_nei"] = nc.dram_tensor("x_nei", [bc, K * D], F32, kind="ExternalInput").ap()
    io["bpack"] = nc.dram_tensor("bpack", [P, BPACK_COLS], BF16, kind="ExternalInput").ap()
    io["qpack"] = nc.dram_tensor("qpack", [P, QPACK_COLS], FP8, kind="ExternalInput").ap()
    io["fpack"] = nc.dram_tensor("fpack", [P, FPACK_COLS], F32, kind="ExternalInput").ap()
    io["vpack"] = nc.dram_tensor("vpack", [1, VPACK_COLS], BF16, kind="ExternalInput").ap()
    io["out"] = nc.dram_tensor("out", [bc, D], F32, kind="ExternalOutput").ap()
    with tile.TileContext(nc) as tc:
        emit_layer(tc, io, n_blocks)
    nc.compile()
    return nc


_CACHED_NC = None


def make_in_maps(inputs):
    import ml_dtypes
    f = np.float32
    bf = ml_dtypes.bfloat16
    f8 = ml_dtypes.float8_e4m3
    x_anc = np.ascontiguousarray(inputs["x_anc"], dtype=f)
    x_nei = np.ascontiguousarray(inputs["x_nei"], dtype=f)
    ln1_g = np.asarray(inputs["ln1_g"], f)
    ln1_b = np.asarray(inputs["ln1_b"], f)
    ln2_g = np.asarray(inputs["ln2_g"], f)
    ln2_b = np.asarray(inputs["ln2_b"], f)
    sc = f(1.0 / math.sqrt(DK))

    # fold LN gains + score scale + fp8 weight prescale into matmul weights
    wq_f = (ln1_g[:, None] * np.asarray(inputs["W_q"], f)) * (sc / WS)
    bias_q = (ln1_b @ np.asarray(inputs["W_q"], f) * (sc / WS))[None, :]
    wo_f = np.asarray(inputs["W_o"], f) / WS
    ff1w_f = ln2_g[:, None] * np.asarray(inputs["ff1_w"], f)
    bias_ff1 = (np.asarray(inputs["ff1_b"], f)
                + ln2_b @ np.asarray(inputs["ff1_w"], f))[None, :]
    bias_ff2 = np.asarray(inputs["ff2_b"], f)[None, :]

    def kt_pack(w, ncols):  # [kt*128, n] -> [128, kt*n]
        ktn = w.shape[0] // P
        return w.reshape(ktn, P, ncols).transpose(1, 0, 2).reshape(P, ktn * ncols)

    bpack = np.concatenate([
        kt_pack(wq_f, D), kt_pack(wo_f, D), kt_pack(ff1w_f, FF),
        kt_pack(np.asarray(inputs["ff2_w"], f), D),
        np.eye(P, dtype=f),
    ], axis=1).astype(bf)
    wkv = np.concatenate([WS * np.asarray(inputs["W_k"], f),
                          WS * np.asarray(inputs["W_v"], f)], axis=1)  # [256, 512]
    qpack = kt_pack(wkv, 2 * D).astype(f8)       # [128, (ko=2, 512)]
    fpack = np.broadcast_to(bias_q, (P, D)).astype(f).copy()
    vpack = np.concatenate([
        np.ones((1, P), f), bias_ff1, bias_ff2], axis=1).astype(bf)
    shared = {
        "bpack": np.ascontiguousarray(bpack),
        "qpack": np.ascontiguousarray(qpack),
        "fpack": np.ascontiguousarray(fpack),
        "vpack": np.ascontiguousarray(vpack),
    }
    in_maps = []
    for c in range(NCORES):
        sl = slice(c * BC, (c + 1) * BC)
        m = dict(shared)
        m["x_anc"] = np.ascontiguousarray(x_anc[sl])
        m["x_nei"] = np.ascontiguousarray(x_nei[sl].reshape(BC, K * D))
        in_maps.append(m)
    return in_maps


def get_nc():
    global _CACHED_NC
    if _CACHED_NC is None:
        _CACHED_NC = build_bass()
    return _CACHED_NC


def kernel(**inputs):
    from concourse.bass_utils import run_bass_kernel_spmd

    in_maps = make_in_maps(inputs)
    res = run_bass_kernel_spmd(get_nc(), in_maps, core_ids=list(range(NCORES)))
    return np.concatenate([res.results[c]["out"] for c in range(NCORES)], axis=0)


# revision 3
# speedup vs baseline: 1.2255x; 1.2066x over previous
"""Trainium2 Bass kernel v2 for nn_NoGraphLayer (single-query neighbor attention + FFN).

Full inputs in, full output out; data-parallel B across 8 cores (4096 anchors each,
32 blocks of 128).

v2 strategy (vs fp32r baseline):
  - x_nei is cast fp32->bf16 during the HBM load (gpsimd SWDGE casting DMA), so
    PE transposes run in bf16 (FWL weight loads, 1 cyc/row).
  - K and V projections fused into ONE DoubleRow fp8 matmul per neighbor
    (contraction 256 in a single pass). W_k/W_v are 16x-scaled host-side so fp8
    avoids the subnormal range; 1/16 folded into W_q (K side) and W_o (V side).
  - Transposed x tiles are copied PSUM->SBUF as fp8e4 in the DR-interleaved
    layout [ki, k, ko, b] by ACT (the copy that had to happen anyway).
  - DVE softmax path in bf16 with layouts keeping a packed innermost dim:
    scores/e in [p,(h,k)], Vv/pv in [p,(h dk,k)] so e-broadcast multiplies and
    k-reductions stay 2x/4x eligible.
  - attn@V summed on DVE (contiguous inner-k reduce), not PE identity-matmuls.
  - Vv drained PSUM->SBUF bf16 by ACT per 4-k group; Kv consumed directly from
    PSUM by the scores multiply (DVE), so kv PSUM tiles free quickly.
  - LayerNorm gains/biases folded into adjacent weights; rsqrt via exp(-.5 ln .)
    (resident act table); gelu keeps the exact ACT Gelu.
"""

import math
from contextlib import ExitStack

import numpy as np

import concourse.bass as bass
import concourse.tile as tile
from concourse import bacc
from concourse import mybir

F32 = mybir.dt.float32
BF16 = mybir.dt.bfloat16
FP8 = mybir.dt.float8e4
AX = mybir.AxisListType
ALU = mybir.AluOpType
ACT = mybir.ActivationFunctionType
DR = mybir.MatmulPerfMode.DoubleRow

B, K, D, H, FF = 32768, 32, 256, 8, 1024
DK = D // H
P = 128
NCORES = 8
BC = B // NCORES
NBLK = BC // P
EPS = 1e-5
WS = 16.0                      # host-side scale on W_k/W_v for fp8 range
KG = 4                         # k's per kv psum tile / drain group
TB = 8                         # transposes per psum batch (= 4 k's)

# const pack layouts (free-dim columns)
BPACK_COLS = 2 * D + 2 * D + 2 * FF + 8 * D + P      # wq, wo, f1w, f2w, identb
FPACK_COLS = D                                        # bq replicated
VPACK_COLS = P + FF + D                               # ones, b1, b2 (bf16 rows)
QPACK_COLS = 2 * 2 * D                                # wkv fp8 [2, 512]


def emit_layer(tc, io, n_blocks):
    nc = tc.nc
    with ExitStack() as ctx:
        ctx.enter_context(nc.allow_low_precision(
            reason="bf16 softmax path is within the 2e-2 gate"))
        const = ctx.enter_context(tc.tile_pool(name="const", bufs=1))
        xpool = ctx.enter_context(tc.tile_pool(name="xin", bufs=2))
        xtp = ctx.enter_context(tc.tile_pool(name="xt", bufs=2))
        work = ctx.enter_context(tc.tile_pool(name="work", bufs=2))
        big = ctx.enter_context(tc.tile_pool(name="big", bufs=2))
        t1p = ctx.enter_context(tc.tile_pool(name="t1p", bufs=1))
        vvp = ctx.enter_context(tc.tile_pool(name="vvp", bufs=2))
        ps_t = ctx.enter_context(tc.tile_pool(name="pst", bufs=2, space="PSUM"))
        ps_kv = ctx.enter_context(tc.tile_pool(name="pskv", bufs=1, space="PSUM"))
        ps_f = ctx.enter_context(tc.tile_pool(name="psf", bufs=2, space="PSUM"))

        bpack = const.tile([P, BPACK_COLS], BF16, tag="c_bpack")
        nc.sync.dma_start(bpack[:], io["bpack"])
        qpack = const.tile([P, QPACK_COLS], FP8, tag="c_qpack")
        nc.sync.dma_start(qpack[:], io["qpack"])
        fpack = const.tile([P, FPACK_COLS], F32, tag="c_fpack")
        nc.sync.dma_start(fpack[:], io["fpack"])
        vpack = const.tile([1, VPACK_COLS], BF16, tag="c_vpack")
        nc.sync.dma_start(vpack[:], io["vpack"])

        off = [0]

        def bslice(n, view=None):
            ap = bpack[:, off[0]:off[0] + n]
            off[0] += n
            return ap if view is None else ap.rearrange("p (kt n) -> p kt n", n=view)

        wq = bslice(2 * D, D)          # [p, 2, 256]
        wo = bslice(2 * D, D)
        f1w = bslice(2 * FF, FF)       # [p, 2, 1024]
        f2w = bslice(8 * D, D)         # [p, 8, 256]
        identb = bslice(P)
        wkv = qpack[:].rearrange("p (ko n) -> p ko n", n=2 * D)  # [p, 2, 512] fp8
        bq = fpack[:, 0:D]             # [128, 256] f32 replicated
        ones1 = vpack[:, 0:P]
        b1 = vpack[:, P:P + FF]
        b2 = vpack[:, P + FF:P + FF + D]

        def layernorm_normed(x_t, out_t, tag):
            # out = (x - mu) * rsqrt(var + eps); g/b folded into next matmul
            st6 = work.tile([P, 6], F32, tag=tag + "_st6")
            nc.vector.bn_stats(st6[:], x_t[:])
            mv = work.tile([P, 2], F32, tag=tag + "_mv")
            nc.vector.bn_aggr(mv[:], st6[:])
            ve = work.tile([P, 1], F32, tag=tag + "_ve")
            nc.vector.tensor_scalar(
                out=ve[:], in0=mv[:, 1:2], scalar1=EPS, scalar2=None, op0=ALU.add)
            lnv = work.tile([P, 1], F32, tag=tag + "_lnv")
            nc.scalar.activation(lnv[:], ve[:], ACT.Ln)
            rs = work.tile([P, 1], F32, tag=tag + "_rs")
            nc.scalar.activation(rs[:], lnv[:], ACT.Exp, scale=-0.5)
            nc.vector.tensor_scalar(
                out=out_t[:], in0=x_t[:], scalar1=mv[:, 0:1], scalar2=rs[:],
                op0=ALU.subtract, op1=ALU.mult)

        def transpose_small(src_ap, n128, dst_tile, tag):
            # src [P, n128*128] bf16 SBUF -> dst [P, n128, 128] bf16 SBUF
            tp = ps_t.tile([P, TB * P], BF16, tag="pt")
            for j in range(n128):
                nc.tensor.transpose(
                    tp[:, j * P:(j + 1) * P], src_ap[:, j * P:(j + 1) * P], identb)
            nc.scalar.copy(
                dst_tile[:].rearrange("p a b -> p (a b)"), tp[:, :n128 * P])

        def load_block(i):
            xa = xpool.tile([P, D], F32, tag="xa")
            nc.sync.dma_start(xa[:], io["x_anc"][i * P:(i + 1) * P, :])
            xnb = xpool.tile([P, K * D], BF16, tag="xnb")
            nc.gpsimd.dma_start(xnb[:], io["x_nei"][i * P:(i + 1) * P, :])
            return {"xa": xa, "xnb": xnb}

        def emit_q(st):
            # LN1 + Q -> qs (bf16, bias folded via replicated-row add)
            lnx = work.tile([P, D], BF16, tag="lnx")
            layernorm_normed(st["xa"], lnx, "ln1")
            lnxT = work.tile([P, 2, P], BF16, tag="lnxT")
            transpose_small(lnx[:], 2, lnxT, "lnxT")
            qp = ps_f.tile([P, D], F32, tag="psf")
            for kt in range(2):
                nc.tensor.matmul(qp[:], lnxT[:, kt, :], wq[:, kt, :],
                                 start=(kt == 0), stop=(kt == 1))
            qs = work.tile([P, D], BF16, tag="qs")
            nc.vector.tensor_add(qs[:], qp[:], bq)
            st["qs"] = qs

        def emit_tb(st, tb):
            # transpose batch tb of x_nei (bf16) -> xTil fp8 [ki, k, ko, b]
            if tb == 0:
                xTil = xtp.tile([P, K, 2, P], FP8, tag="xTil")
                st["xTil"] = xTil
            tp = ps_t.tile([P, TB * P], BF16, tag="pt")
            for j in range(TB):
                c = tb * TB + j
                nc.tensor.transpose(
                    tp[:, j * P:(j + 1) * P],
                    st["xnb"][:, c * P:(c + 1) * P], identb)
            ks = tb * (TB // 2)
            nc.scalar.copy(
                st["xTil"][:, ks:ks + TB // 2].rearrange("p a b c -> p (a b c)"),
                tp[:, :TB * P])

        def emit_kv_g(st, g):
            # fused K|V fp8-DR projections for KG k's + scores mul + Vv drain
            if g == 0:
                pr = big.tile([P, K, D], BF16, tag="pr")
                vvb = vvp.tile([P, K, D], BF16, tag="vvb")
                st["pr"], st["vvb"] = pr, vvb
            kvp = ps_kv.tile([P, KG, 2 * D], F32, tag="pskv")
            for j in range(KG):
                k = g * KG + j
                nc.tensor.matmul(kvp[:, j, :], st["xTil"][:, k], wkv,
                                 start=True, stop=True, perf_mode=DR)
            nc.vector.tensor_mul(
                st["pr"][:, g * KG:(g + 1) * KG, :],
                kvp[:, :, 0:D],
                st["qs"][:].rearrange("p (o n) -> p o n", o=1)
                    .to_broadcast((P, KG, D)))
            nc.scalar.copy(st["vvb"][:, g * KG:(g + 1) * KG, :],
                           kvp[:, :, D:2 * D])

        def emit_bc1(st, i):
            pr, vvb, xa = st["pr"], st["vvb"], st["xa"]
            # ---- dk-reduce via TT tree (tensor_reduce is 1.3ns/elem on HW) ----
            pr4 = pr[:].rearrange("p k (h dk) -> p k h dk", h=H)
            t1 = t1p.tile([P, K, H, 16], BF16, tag="t1")
            nc.vector.tensor_add(t1[:], pr4[:, :, :, 0:16], pr4[:, :, :, 16:32])
            t2 = work.tile([P, K, H, 8], BF16, tag="t2")
            nc.vector.tensor_add(t2[:], t1[:, :, :, 0:8], t1[:, :, :, 8:16])
            t3 = work.tile([P, K, H, 4], BF16, tag="t3")
            nc.vector.tensor_add(t3[:], t2[:, :, :, 0:4], t2[:, :, :, 4:8])
            t4 = work.tile([P, K, H, 2], BF16, tag="t4")
            nc.vector.tensor_add(t4[:], t3[:, :, :, 0:2], t3[:, :, :, 2:4])
            scoresN = work.tile([P, K, H], BF16, tag="scores")
            nc.vector.tensor_add(
                scoresN[:].rearrange("p k (h o) -> p k h o", o=1),
                t4[:, :, :, 0:1], t4[:, :, :, 1:2])
            e = work.tile([P, K, H], BF16, tag="e")
            nc.scalar.activation(
                e[:].rearrange("p k h -> p (k h)"),
                scoresN[:].rearrange("p k h -> p (k h)"), ACT.Exp)
            z = work.tile([P, H], F32, tag="z")
            nc.vector.tensor_reduce(
                z[:], e[:].rearrange("p k h -> p h k"), axis=AX.X, op=ALU.add)
            zr = work.tile([P, H], F32, tag="zr")
            nc.vector.reciprocal(zr[:], z[:])

            # ---- attn @ V: pv = e*Vv (bcast over dk), k-reduce on PE ----
            pv = vvp.tile([P, K, D], BF16, tag="pv")
            nc.vector.tensor_mul(
                pv[:].rearrange("p k (h dk) -> p k h dk", h=H),
                vvb[:].rearrange("p k (h dk) -> p k h dk", h=H),
                e[:].rearrange("p k (h o) -> p k h o", o=1)
                    .to_broadcast((P, K, H, DK)))
            upp = ps_kv.tile([P, KG, 2 * D], F32, tag="pskv")
            for k in range(K):
                nc.tensor.matmul(upp[:, 0, 0:D], identb, pv[:, k, :],
                                 start=(k == 0), stop=(k == K - 1))
            upn = work.tile([P, D], BF16, tag="upn")
            nc.vector.tensor_mul(
                upn[:].rearrange("p (h dk) -> p h dk", h=H),
                upp[:, 0, 0:D].rearrange("p (h dk) -> p h dk", h=H),
                zr[:].rearrange("p (h o) -> p h o", o=1).to_broadcast((P, H, DK)))

            # ---- W_o + residual ----
            uT = work.tile([P, 2, P], BF16, tag="uT")
            transpose_small(upn[:], 2, uT, "uT")
            aop = ps_f.tile([P, D], F32, tag="psf")
            for kt in range(2):
                nc.tensor.matmul(aop[:], uT[:, kt, :], wo[:, kt, :],
                                 start=(kt == 0), stop=(kt == 1))
            xs = work.tile([P, D], F32, tag="xs")
            nc.vector.tensor_add(xs[:], aop[:], xa[:])

            # ---- LN2 + FF ----
            hs = work.tile([P, D], BF16, tag="hs")
            layernorm_normed(xs, hs, "ln2")
            hT = work.tile([P, 2, P], BF16, tag="hT")
            transpose_small(hs[:], 2, hT, "hT")
            st["hT"], st["xs"] = hT, xs

        def emit_bc2(st, i):
            hT, xs = st["hT"], st["xs"]
            ffg = work.tile([P, FF], BF16, tag="ffg")
            for nh in range(2):
                fp = ps_f.tile([P, 512], F32, tag="psf")
                for kt in range(2):
                    nc.tensor.matmul(fp[:], hT[:, kt, :],
                                     f1w[:, kt, nh * 512:(nh + 1) * 512],
                                     start=(kt == 0), stop=False)
                nc.tensor.matmul(fp[:], ones1, b1[:, nh * 512:(nh + 1) * 512],
                                 start=False, stop=True)
                nc.scalar.activation(ffg[:, nh * 512:(nh + 1) * 512], fp[:],
                                     ACT.Gelu)
            fgT = work.tile([P, 8, P], BF16, tag="fgT")
            for q in range(2):
                transpose_small(ffg[:, q * 512:(q + 1) * 512], 4,
                                fgT[:, q * 4:(q + 1) * 4], "fgT")
            f2p = ps_f.tile([P, D], F32, tag="psf")
            for kt in range(8):
                nc.tensor.matmul(f2p[:], fgT[:, kt, :], f2w[:, kt, :],
                                 start=(kt == 0), stop=False)
            nc.tensor.matmul(f2p[:], ones1, b2, start=False, stop=True)
            outs = work.tile([P, D], F32, tag="outs")
            nc.vector.tensor_add(outs[:], f2p[:], xs[:])
            nc.sync.dma_start(io["out"][i * P:(i + 1) * P, :], outs[:])

        # ---- software-pipelined emission (engines execute in-order, so the
        # next block's transposes are interleaved into this block's KV groups
        # to fill PE stalls while PSUM kv tiles drain / softmax runs) ----
        sts = {0: load_block(0)}
        emit_q(sts[0])
        for tb in range(K * 2 // TB):
            emit_tb(sts[0], tb)
        for i in range(n_blocks):
            if i + 1 < n_blocks:
                sts[i + 1] = load_block(i + 1)
            for g in range(K // KG):
                emit_kv_g(sts[i], g)
                if i + 1 < n_blocks:
                    emit_tb(sts[i + 1], g)
            if i - 1 in sts:
                emit_bc2(sts[i - 1], i - 1)   # prev block's FFN fills PE
                del sts[i - 1]
            if i + 1 < n_blocks:
                emit_q(sts[i + 1])
            emit_bc1(sts[i], i)
        emit_bc2(sts[n_blocks - 1], n_blocks - 1)
        del sts[n_blocks - 1]


_ACT_TABLES_PATCHED = False


def _patch_act_tables():
    # Bias bacc's act-table chooser: Ln and Exp both resolve to
    # natural_log_exp_and_others (one resident table for LN-rsqrt and softmax);
    # only Gelu swaps tables.
    global _ACT_TABLES_PATCHED
    if _ACT_TABLES_PATCHED:
        return
    import concourse.bacc as _bacc_mod
    _orig = _bacc_mod.get_activation_tables

    def patched(arch):
        t = dict(_orig(arch))
        exp_t = mybir.ActivationFunctionType.Exp
        ln_t = mybir.ActivationFunctionType.Ln
        for name, fns in t.items():
            if name != "natural_log_exp_and_others" and (
                    exp_t in fns or ln_t in fns):
                t[name] = fns - {exp_t, ln_t}
        return t

    _bacc_mod.get_activation_tables = patched
    _ACT_TABLES_PATCHED = True


def build_bass(n_blocks=NBLK, bc=BC):
    _patch_act_tables()
    nc = bacc.Bacc("TRN2", target_bir_lowering=False, debug=False,
                   num_devices=NCORES)
    io = {}
    io["x_anc"] = nc.dram_tensor("x_anc", [bc, D], F32, kind="ExternalInput").ap()
    io["x_nei"] = nc.dram_tensor("x_nei", [bc, K * D], F32, kind="ExternalInput").ap()
    io["bpack"] = nc.dram_tensor("bpack", [P, BPACK_COLS], BF16, kind="ExternalInput").ap()
    io["qpack"] = nc.dram_tensor("qpack", [P, QPACK_COLS], FP8, kind="ExternalInput").ap()
    io["fpack"] = nc.dram_tensor("fpack", [P, FPACK_COLS], F32, kind="ExternalInput").ap()
    io["vpack"] = nc.dram_tensor("vpack", [1, VPACK_COLS], BF16, kind="ExternalInput").ap()
    io["out"] = nc.dram_tensor("out", [bc, D], F32, kind="ExternalOutput").ap()
    with tile.TileContext(nc) as tc:
        emit_layer(tc, io, n_blocks)
    nc.compile()
    return nc


_CACHED_NC = None


def make_in_maps(inputs):
    import ml_dtypes
    f = np.float32
    bf = ml_dtypes.bfloat16
    f8 = ml_dtypes.float8_e4m3
    x_anc = np.ascontiguousarray(inputs["x_anc"], dtype=f)
    x_nei = np.ascontiguousarray(inputs["x_nei"], dtype=f)
    ln1_g = np.asarray(inputs["ln1_g"], f)
    ln1_b = np.asarray(inputs["ln1_b"], f)
    ln2_g = np.asarray(inputs["ln2_g"], f)
    ln2_b = np.asarray(inputs["ln2_b"], f)
    sc = f(1.0 / math.sqrt(DK))

    # fold LN gains + score scale + fp8 weight prescale into matmul weights
    wq_f = (ln1_g[:, None] * np.asarray(inputs["W_q"], f)) * (sc / WS)
    bias_q = (ln1_b @ np.asarray(inputs["W_q"], f) * (sc / WS))[None, :]
    wo_f = np.asarray(inputs["W_o"], f) / WS
    ff1w_f = ln2_g[:, None] * np.asarray(inputs["ff1_w"], f)
    bias_ff1 = (np.asarray(inputs["ff1_b"], f)
                + ln2_b @ np.asarray(inputs["ff1_w"], f))[None, :]
    bias_ff2 = np.asarray(inputs["ff2_b"], f)[None, :]

    def kt_pack(w, ncols):  # [kt*128, n] -> [128, kt*n]
        ktn = w.shape[0] // P
        return w.reshape(ktn, P, ncols).transpose(1, 0, 2).reshape(P, ktn * ncols)

    bpack = np.concatenate([
        kt_pack(wq_f, D), kt_pack(wo_f, D), kt_pack(ff1w_f, FF),
        kt_pack(np.asarray(inputs["ff2_w"], f), D),
        np.eye(P, dtype=f),
    ], axis=1).astype(bf)
    wkv = np.concatenate([WS * np.asarray(inputs["W_k"], f),
                          WS * np.asarray(inputs["W_v"], f)], axis=1)  # [256, 512]
    qpack = kt_pack(wkv, 2 * D).astype(f8)       # [128, (ko=2, 512)]
    fpack = np.broadcast_to(bias_q, (P, D)).astype(f).copy()
    vpack = np.concatenate([
        np.ones((1, P), f), bias_ff1, bias_ff2], axis=1).astype(bf)
    shared = {
        "bpack": np.ascontiguousarray(bpack),
        "qpack": np.ascontiguousarray(qpack),
        "fpack": np.ascontiguousarray(fpack),
        "vpack": np.ascontiguousarray(vpack),
    }
    in_maps = []
    for c in range(NCORES):
        sl = slice(c * BC, (c + 1) * BC)
        m = dict(shared)
        m["x_anc"] = np.ascontiguousarray(x_anc[sl])
        m["x_nei"] = np.ascontiguousarray(x_nei[sl].reshape(BC, K * D))
        in_maps.append(m)
    return in_maps


def get_nc():
    global _CACHED_NC
    if _CACHED_NC is None:
        _CACHED_NC = build_bass()
    return _CACHED_NC


def kernel(**inputs):
    from concourse.bass_utils import run_bass_kernel_spmd

    in_maps = make_in_maps(inputs)
    res = run_bass_kernel_spmd(get_nc(), in_maps, core_ids=list(range(NCORES)))
    return np.concatenate([res.results[c]["out"] for c in range(NCORES)], axis=0)
